# revision 42
# baseline (speedup 1.0000x reference)
"""TRN2 Bass kernel for OneLayerCNN: conv2d(4x4, stride 2, pad 2) + bias + ReLU.

Input  A_prev (64, 256, 256, 3) f32, W (4,4,3,16), b (1,1,1,16)
Output (64, 129*129*16) f32.

Data-parallel over 8 NeuronCores (8 images each). Per core, per h-block of
15 output rows:
  - row-pair tiles: partition (re, img) holds input rows (2re, 2re+1) as one
    6KB contiguous DMA descriptor; parities are column halves of the tile.
  - PE transposes (is_transpose matmul vs identity) turn [instance, offset]
    windows into [offset, instance] SBUF tiles (contraction on partitions).
  - conv = per w-block 4 accumulating float32r matmuls: stationary lhsT =
    transposed-activation window [K<=121, M=120 instances], moving rhs =
    host-precomputed banded weights [121, 304 = 19 w' x 16 cout]. The bias
    rides a ones-row at K=kw on the fh=0 matmul; border w-blocks use
    K-truncated windows with host-shifted weight variants (no zero padding).
  - fused ReLU on PSUM eviction (DVE/ACT), one contiguous-run output DMA
    per h-block (8KB+ descriptors).
A post-pass splits multi-sem-wait instructions (this walrus accepts one
sync wait per instruction). A short PE warmup during the initial DMA wait
opens the HAM clock gate before the real matmuls.
"""
import numpy as np
from contextlib import ExitStack

import concourse.bass as bass
import concourse.tile as tile
from concourse import mybir
from concourse.bass_utils import run_bass_kernel_spmd
import bass_rust

# ---------------- problem constants (hardcoded) ----------------
N_CORES = 8
IMG = 8              # images per core
H = 256
WID = 256
CIN = 3
F = 4
COUT = 16
HO = 129
WO = 129
RW = WID * CIN       # 768 floats per row
NH_FULL = 15         # h' rows per full block
NB = 9               # 8 full blocks + 1 ragged (9 h')
WBLK = 19            # w' per w-block (B=0..5), B=6 computes 16, keeps 15
NWB = 7
KW = 120             # banded K window (6*18+12)
KB = 121             # K incl bias row
NMM = WBLK * COUT    # 304
OUTROW = WO * COUT   # 2064

DT_MM = mybir.dt.float32r   # matmul dtype knob (float32r | float32)
DT_F32 = mybir.dt.float32


def _split_multi_waits(nc):
    """walrus here accepts at most ONE sync wait per instruction; hoist
    extras onto NoOps inserted just before, same engine queue."""
    ctr = 0
    for f in nc.m.functions:
        for bb in f.blocks:
            insts = bb.instructions  # live list
            out = []
            changed = False
            for inst in insts:
                si = inst.sync_info
                if si is None:
                    out.append(inst)
                    continue
                waits = list(si.on_wait)
                if len(waits) > 1:
                    changed = True
                    for w in waits[:-1]:
                        ctr += 1
                        nop = mybir.InstNoOp(name=f"I-wsplit-{ctr}")
                        nop.engine = inst.engine
                        nop.sync_info = bass_rust.SyncInfo(
                            on_wait=[w], on_update=[])
                        out.append(nop)
                    inst.sync_info = bass_rust.SyncInfo(
                        on_wait=[waits[-1]], on_update=list(si.on_update))
                out.append(inst)
            if changed:
                insts[:] = out
    return nc


def _make_wband(W_arr, b_arr):
    """4 banded weight mats [121, 304]: wb[fh][6s+3fw+ci, 16s+co] = W[fh,fw,ci,co];
    wb[0][120, 16s+co] = b[co]. Plus two bias-carrying edge variants for the
    first/last w-blocks whose K windows are truncated at the image border:
    wb0_e0 [115, 304] = [wb0[6:120]; bias], wb0_e6 [91, 256] = [wb0[0:90]; bias]."""
    wbs = []
    for fh in range(F):
        wb = np.zeros((KB, NMM), dtype=np.float32)
        for s in range(WBLK):
            for fw in range(F):
                for ci in range(CIN):
                    wb[6 * s + 3 * fw + ci, 16 * s:16 * s + 16] = \
                        W_arr[fh, fw, ci, :]
        if fh == 0:
            for s in range(WBLK):
                wb[120, 16 * s:16 * s + 16] = b_arr.reshape(-1)
        wbs.append(wb)
    # B=0 edge: all four taps shifted to row 0 (SBUF operands must start at
    # an aligned base partition, so slices [6:120] are precomputed on host)
    e0 = [np.concatenate([wbs[0][6:120], wbs[0][120:121]], axis=0)]  # [115,304]
    for fh in range(1, F):
        e0.append(wbs[fh][6:120].copy())                             # [114,304]
    e6 = np.concatenate([wbs[0][0:90, 0:240],
                         wbs[0][120:121, 0:240]], axis=0)            # [91,240]
    # pack weights into one [128, CONSTW] tensor (one DMA); identity ships
    # separately so the first transposes are not gated on the big transfer
    blocks = e0 + wbs + [e6]
    comb = np.zeros((128, sum(a.shape[1] for a in blocks)), dtype=np.float32)
    col = 0
    for a in blocks:
        comb[0:a.shape[0], col:col + a.shape[1]] = a
        col += a.shape[1]
    return comb, np.eye(128, dtype=np.float32)


def _build_nc(dt_mm=DT_MM):
    nc = bass.Bass()
    a_in = nc.declare_dram_parameter("A", [IMG, H, RW], dt_mm, isOutput=False)
    CONSTW = NMM * 8 + 240
    c_in = nc.declare_dram_parameter("consts", [128, CONSTW], dt_mm,
                                     isOutput=False)
    id_in = nc.declare_dram_parameter("ident", [128, 128], dt_mm,
                                      isOutput=False)
    z_out = nc.declare_dram_parameter("Z", [IMG, HO, OUTROW], DT_F32,
                                      isOutput=True)

    # row pairs: apair[re][img] = rows (2re, 2re+1) concatenated, 6KB each
    apair = a_in.rearrange("i (re two) c -> re i (two c)", two=2)

    # per-w-block geometry: window start col, K width, matmul N, evicted N
    #   B=0 and B=6 have border-truncated K windows (no zero padding needed;
    #   dropped K rows correspond exactly to the conv's zero pads)
    BGEO = []
    for B in range(NWB):
        win = max(0, 114 * B - 6)
        kw = min(RW, 114 * B - 6 + KW) - win     # 114 | 120 | 90
        nmm = NMM if B < 6 else 240              # B=6: 15 w' (no junk cols;
        ncols = NMM if B < 6 else 240            # f32r has no N<256 penalty)
        BGEO.append((win, kw, nmm, ncols))

    with tile.TileContext(nc) as tc, ExitStack() as ctx:
        consts = ctx.enter_context(tc.tile_pool(name="consts", bufs=1))
        rpool = ctx.enter_context(tc.tile_pool(name="rows", bufs=4))
        tpool = ctx.enter_context(tc.tile_pool(name="tsb", bufs=6))
        opool = ctx.enter_context(tc.tile_pool(name="oacc", bufs=2))
        pt_pool = ctx.enter_context(
            tc.tile_pool(name="ptr", bufs=4, space="PSUM"))
        pc_pool = ctx.enter_context(
            tc.tile_pool(name="pconv", bufs=3, space="PSUM"))
        pw_pool = ctx.enter_context(
            tc.tile_pool(name="pwarm", bufs=1, space="PSUM"))

        # PE warmup: ~5us of dummy matmuls during the initial DMA wait so the
        # HAM clock gate opens (1.2 -> 2.4 GHz) before the real work starts
        wtile = consts.tile([128, 640], dt_mm, tag="wtile", name="wtile")
        nc.gpsimd.memset(wtile[:].bitcast(DT_F32), 0.0)
        pwarm = pw_pool.tile([128, 512], DT_F32, tag="pwarm", name="pwarm")
        for _ in range(6):
            nc.tensor.matmul(pwarm[:], wtile[0:128, 0:128],
                             wtile[0:128, 128:640], start=True, stop=True)

        # identity first (tiny, gates the first transposes), then one DMA
        # for the weight set; both on the scalar HWDGE queue so the row
        # loads (sync queue) are not stuck behind them
        ident = consts.tile([128, 128], dt_mm, tag="ident", name="ident")
        nc.scalar.dma_start(out=ident[:], in_=id_in[:])
        call = consts.tile([128, CONSTW], dt_mm, tag="call", name="call")
        E0W = NMM * 4
        # two chunks: B=0 edge weights first (gate the first conv matmuls)
        nc.scalar.dma_start(out=call[:, 0:E0W], in_=c_in[:, 0:E0W])
        nc.scalar.dma_start(out=call[:, E0W:CONSTW], in_=c_in[:, E0W:CONSTW])
        off = 0
        wbe0 = []
        for fh in range(F):
            wbe0.append(call[0:(115 if fh == 0 else 114), off:off + NMM])
            off += NMM
        wb_sb = []
        for fh in range(F):
            wb_sb.append(call[0:KB, off:off + NMM])
            off += NMM
        wbe6 = call[0:91, off:off + 240]
        ident_ap = ident[:]

        for b in range(NB):
            h0 = NH_FULL * b
            nh = NH_FULL if b < NB - 1 else HO - NH_FULL * (NB - 1)  # 15 | 9
            nl = nh + 1          # parity rows needed: re = h0-1 .. h0+nh-1
            m = nh * IMG         # matmul M (120 | 72)

            # one tile holds both parities: partition (re,img) = rows
            # (2re, 2re+1) back to back -> one 6KB descriptor per partition
            rp = rpool.tile([128, 2 * RW], dt_mm, tag="rp", name="rp")
            l0, l1 = 0, nl
            if b == 0:
                l0 = 1                      # re = -1 is a zero row pair
                nc.gpsimd.memset(rp[0:8, :].bitcast(DT_F32), 0.0)
            if b == NB - 1:
                l1 = nl - 1                 # re = 128 is a zero row pair
                # 32-aligned base; rows below (nl-1)*8 are re-loaded by
                # the DMA below, which follows in program order (WAW)
                nc.gpsimd.memset(rp[64:128, :].bitcast(DT_F32), 0.0)
            re0 = h0 - 1 + l0
            asrc = apair[re0:re0 + (l1 - l0)]
            if b == 0:
                # stage the B=0 windows (both parities) first so the first
                # transposes can start as early as possible
                nc.sync.dma_start(out=rp[l0 * 8:l1 * 8, 0:128],
                                  in_=asrc[:, :, 0:128])
                nc.sync.dma_start(out=rp[l0 * 8:l1 * 8, 768:896],
                                  in_=asrc[:, :, 768:896])
                nc.sync.dma_start(out=rp[l0 * 8:l1 * 8, 128:768],
                                  in_=asrc[:, :, 128:768])
                nc.sync.dma_start(out=rp[l0 * 8:l1 * 8, 896:2 * RW],
                                  in_=asrc[:, :, 896:2 * RW])
            else:
                nc.sync.dma_start(out=rp[l0 * 8:l1 * 8, :], in_=asrc)

            oacc = opool.tile([128, OUTROW], DT_F32, tag="oacc")
            for B in range(NWB):
                win, kw, nmm, ncols = BGEO[B]
                # rhs weights for the four fh taps (B=0/6 use row slices)
                r0 = win - (114 * B - 6)         # 6 at B=0 else 0
                if B == 0:
                    wrhs0 = wbe0[0][0:kw + 1, 0:nmm]
                    wrhs = [wbe0[fh][0:kw, 0:nmm] for fh in range(1, F)]
                elif B == 6:
                    wrhs0 = wbe6[0:kw + 1, 0:nmm]
                    wrhs = [wb_sb[fh][0:kw, 0:nmm] for fh in range(1, F)]
                else:
                    wrhs0 = wb_sb[0][0:kw + 1, 0:nmm]
                    wrhs = [wb_sb[fh][0:kw, 0:nmm] for fh in range(1, F)]
                ones_base = (kw // 32) * 32      # 32-aligned memset base
                tsb = []
                for p in range(2):
                    ptr = pt_pool.tile([kw, 128], dt_mm, tag="ptr", name="ptr")
                    nc.tensor.transpose(
                        ptr[:], rp[:, RW * p + win:RW * p + win + kw],
                        ident_ap)
                    t = tpool.tile([KB, 128], dt_mm, tag=f"t{p}", name=f"tsb{p}")
                    if p == 0:
                        # ones row at partition kw (bias): memset a 32-aligned
                        # range; the evict below overwrites rows < kw
                        nc.gpsimd.memset(
                            t[ones_base:kw + 1, :].bitcast(DT_F32), 1.0)
                    nc.vector.tensor_copy(t[0:kw, :], ptr[:])
                    tsb.append(t)
                pc = pc_pool.tile([128, NMM], DT_F32, tag="pc")
                nc.tensor.matmul(pc[0:m, 0:nmm], tsb[0][0:kw + 1, 0:m],
                                 wrhs0, start=True, stop=False)
                nc.tensor.matmul(pc[0:m, 0:nmm], tsb[1][0:kw, 0:m],
                                 wrhs[0], start=False, stop=False)
                nc.tensor.matmul(pc[0:m, 0:nmm], tsb[0][0:kw, 8:8 + m],
                                 wrhs[1], start=False, stop=False)
                nc.tensor.matmul(pc[0:m, 0:nmm], tsb[1][0:kw, 8:8 + m],
                                 wrhs[2], start=False, stop=True)
                # ReLU eviction: alternate ACT/DVE to balance engines
                if B % 3 == 2:
                    nc.scalar.activation(
                        oacc[0:m, 304 * B:304 * B + ncols], pc[0:m, 0:ncols],
                        mybir.ActivationFunctionType.Relu)
                else:
                    nc.vector.tensor_scalar_max(
                        oacc[0:m, 304 * B:304 * B + ncols],
                        pc[0:m, 0:ncols], 0.0)

            dst = z_out[:, h0:h0 + nh, :].rearrange("i j c -> j i c")
            nc.scalar.dma_start(out=dst, in_=oacc[0:m, :])

    _split_multi_waits(nc)
    return nc


_NC_CACHE = {}


def _get_nc(dt_mm=DT_MM):
    key = str(dt_mm)
    if key not in _NC_CACHE:
        _NC_CACHE[key] = _build_nc(dt_mm)
    return _NC_CACHE[key]


def kernel(A_prev, W, b, _trace=False, _dt=None):
    A_prev = np.ascontiguousarray(A_prev, dtype=np.float32)
    W = np.asarray(W, dtype=np.float32)
    b = np.asarray(b, dtype=np.float32)
    comb, ident = _make_wband(W, b)

    nc = _get_nc(_dt or DT_MM)
    in_maps = []
    for c in range(N_CORES):
        shard = A_prev[c * IMG:(c + 1) * IMG].reshape(IMG, H, RW)
        in_maps.append({"A": shard, "consts": comb, "ident": ident})

    res = run_bass_kernel_spmd(nc, in_maps, list(range(N_CORES)),
                               trace=_trace)
    out = np.concatenate([res.results[c]["Z"].reshape(IMG, -1)
                          for c in range(N_CORES)], axis=0)
    if _trace:
        return out, res
    return out


# revision 43
# speedup vs baseline: 1.0349x; 1.0349x over previous
"""TRN2 Bass kernel for OneLayerCNN: conv2d(4x4, stride 2, pad 2) + bias + ReLU.

Input  A_prev (64, 256, 256, 3) f32, W (4,4,3,16), b (1,1,1,16)
Output (64, 129*129*16) f32.

Data-parallel over 8 NeuronCores (8 images each). Per core, per h-block of
15 output rows:
  - row-pair tiles: partition (re, img) holds input rows (2re, 2re+1) as one
    6KB contiguous DMA descriptor; parities are column halves of the tile.
  - PE transposes (is_transpose matmul vs identity) turn [instance, offset]
    windows into [offset, instance] SBUF tiles (contraction on partitions).
  - conv = per w-block 4 accumulating float32r matmuls: stationary lhsT =
    transposed-activation window [K<=121, M=120 instances], moving rhs =
    host-precomputed banded weights [121, 304 = 19 w' x 16 cout]. The bias
    rides a ones-row at K=kw on the fh=0 matmul; border w-blocks use
    K-truncated windows with host-shifted weight variants (no zero padding).
  - fused ReLU on PSUM eviction (DVE/ACT), one contiguous-run output DMA
    per h-block (8KB+ descriptors).
A post-pass splits multi-sem-wait instructions (this walrus accepts one
sync wait per instruction). A short PE warmup during the initial DMA wait
opens the HAM clock gate before the real matmuls.
"""
import numpy as np
from contextlib import ExitStack

import concourse.bass as bass
import concourse.tile as tile
from concourse import mybir
from concourse.bass_utils import run_bass_kernel_spmd
import bass_rust

# ---------------- problem constants (hardcoded) ----------------
N_CORES = 8
IMG = 8              # images per core
H = 256
WID = 256
CIN = 3
F = 4
COUT = 16
HO = 129
WO = 129
RW = WID * CIN       # 768 floats per row
NH_FULL = 15         # h' rows per full block
NB = 9               # 8 full blocks + 1 ragged (9 h')
WBLK = 19            # w' per w-block (B=0..5), B=6 computes 16, keeps 15
NWB = 7
KW = 120             # banded K window (6*18+12)
KB = 121             # K incl bias row
NMM = WBLK * COUT    # 304
OUTROW = WO * COUT   # 2064

DT_MM = mybir.dt.float32r   # matmul dtype knob (float32r | float32)
DT_F32 = mybir.dt.float32


def _split_multi_waits(nc):
    """walrus here accepts at most ONE sync wait per instruction; hoist
    extras onto NoOps inserted just before, same engine queue."""
    ctr = 0
    for f in nc.m.functions:
        for bb in f.blocks:
            insts = bb.instructions  # live list
            out = []
            changed = False
            for inst in insts:
                si = inst.sync_info
                if si is None:
                    out.append(inst)
                    continue
                waits = list(si.on_wait)
                if len(waits) > 1:
                    changed = True
                    for w in waits[:-1]:
                        ctr += 1
                        nop = mybir.InstNoOp(name=f"I-wsplit-{ctr}")
                        nop.engine = inst.engine
                        nop.sync_info = bass_rust.SyncInfo(
                            on_wait=[w], on_update=[])
                        out.append(nop)
                    inst.sync_info = bass_rust.SyncInfo(
                        on_wait=[waits[-1]], on_update=list(si.on_update))
                out.append(inst)
            if changed:
                insts[:] = out
    return nc


def _make_wband(W_arr, b_arr):
    """4 banded weight mats [121, 304]: wb[fh][6s+3fw+ci, 16s+co] = W[fh,fw,ci,co];
    wb[0][120, 16s+co] = b[co]. Plus two bias-carrying edge variants for the
    first/last w-blocks whose K windows are truncated at the image border:
    wb0_e0 [115, 304] = [wb0[6:120]; bias], wb0_e6 [91, 256] = [wb0[0:90]; bias]."""
    wbs = []
    for fh in range(F):
        wb = np.zeros((KB, NMM), dtype=np.float32)
        for s in range(WBLK):
            for fw in range(F):
                for ci in range(CIN):
                    wb[6 * s + 3 * fw + ci, 16 * s:16 * s + 16] = \
                        W_arr[fh, fw, ci, :]
        if fh == 0:
            for s in range(WBLK):
                wb[120, 16 * s:16 * s + 16] = b_arr.reshape(-1)
        wbs.append(wb)
    # B=0 edge: all four taps shifted to row 0 (SBUF operands must start at
    # an aligned base partition, so slices [6:120] are precomputed on host)
    e0 = [np.concatenate([wbs[0][6:120], wbs[0][120:121]], axis=0)]  # [115,304]
    for fh in range(1, F):
        e0.append(wbs[fh][6:120].copy())                             # [114,304]
    e6 = np.concatenate([wbs[0][0:90, 0:240],
                         wbs[0][120:121, 0:240]], axis=0)            # [91,240]
    # pack weights into one [128, CONSTW] tensor (one DMA); identity ships
    # separately so the first transposes are not gated on the big transfer
    blocks = e0 + wbs + [e6]
    comb = np.zeros((128, sum(a.shape[1] for a in blocks)), dtype=np.float32)
    col = 0
    for a in blocks:
        comb[0:a.shape[0], col:col + a.shape[1]] = a
        col += a.shape[1]
    return comb, np.eye(128, dtype=np.float32)


def _build_nc(dt_mm=DT_MM):
    nc = bass.Bass()
    a_in = nc.declare_dram_parameter("A", [IMG, H, RW], dt_mm, isOutput=False)
    CONSTW = NMM * 8 + 240
    c_in = nc.declare_dram_parameter("consts", [128, CONSTW], dt_mm,
                                     isOutput=False)
    id_in = nc.declare_dram_parameter("ident", [128, 128], dt_mm,
                                      isOutput=False)
    z_out = nc.declare_dram_parameter("Z", [IMG, HO, OUTROW], DT_F32,
                                      isOutput=True)

    # row pairs: apair[re][img] = rows (2re, 2re+1) concatenated, 6KB each
    apair = a_in.rearrange("i (re two) c -> re i (two c)", two=2)

    # per-w-block geometry: window start col, K width, matmul N, evicted N
    #   B=0 and B=6 have border-truncated K windows (no zero padding needed;
    #   dropped K rows correspond exactly to the conv's zero pads)
    BGEO = []
    for B in range(NWB):
        win = max(0, 114 * B - 6)
        kw = min(RW, 114 * B - 6 + KW) - win     # 114 | 120 | 90
        nmm = NMM if B < 6 else 240              # B=6: 15 w' (no junk cols;
        ncols = NMM if B < 6 else 240            # f32r has no N<256 penalty)
        BGEO.append((win, kw, nmm, ncols))

    with tile.TileContext(nc) as tc, ExitStack() as ctx:
        consts = ctx.enter_context(tc.tile_pool(name="consts", bufs=1))
        rpool = ctx.enter_context(tc.tile_pool(name="rows", bufs=4))
        tpool = ctx.enter_context(tc.tile_pool(name="tsb", bufs=6))
        opool = ctx.enter_context(tc.tile_pool(name="oacc", bufs=2))
        pt_pool = ctx.enter_context(
            tc.tile_pool(name="ptr", bufs=4, space="PSUM"))
        pc_pool = ctx.enter_context(
            tc.tile_pool(name="pconv", bufs=3, space="PSUM"))
        pw_pool = ctx.enter_context(
            tc.tile_pool(name="pwarm", bufs=1, space="PSUM"))

        # PE warmup: ~5us of dummy matmuls during the initial DMA wait so the
        # HAM clock gate opens (1.2 -> 2.4 GHz) before the real work starts
        wtile = consts.tile([128, 640], dt_mm, tag="wtile", name="wtile")
        nc.gpsimd.memset(wtile[:].bitcast(DT_F32), 0.0)
        pwarm = pw_pool.tile([128, 512], DT_F32, tag="pwarm", name="pwarm")
        for _ in range(6):
            nc.tensor.matmul(pwarm[:], wtile[0:128, 0:128],
                             wtile[0:128, 128:640], start=True, stop=True)

        # identity first (tiny, gates the first transposes), then one DMA
        # for the weight set; both on the scalar HWDGE queue so the row
        # loads (sync queue) are not stuck behind them
        ident = consts.tile([128, 128], dt_mm, tag="ident", name="ident")
        nc.scalar.dma_start(out=ident[:], in_=id_in[:])
        call = consts.tile([128, CONSTW], dt_mm, tag="call", name="call")
        nc.scalar.dma_start(out=call[:], in_=c_in[:])
        off = 0
        wbe0 = []
        for fh in range(F):
            wbe0.append(call[0:(115 if fh == 0 else 114), off:off + NMM])
            off += NMM
        wb_sb = []
        for fh in range(F):
            wb_sb.append(call[0:KB, off:off + NMM])
            off += NMM
        wbe6 = call[0:91, off:off + 240]
        ident_ap = ident[:]

        for b in range(NB):
            h0 = NH_FULL * b
            nh = NH_FULL if b < NB - 1 else HO - NH_FULL * (NB - 1)  # 15 | 9
            nl = nh + 1          # parity rows needed: re = h0-1 .. h0+nh-1
            m = nh * IMG         # matmul M (120 | 72)

            # one tile holds both parities: partition (re,img) = rows
            # (2re, 2re+1) back to back -> one 6KB descriptor per partition
            rp = rpool.tile([128, 2 * RW], dt_mm, tag="rp", name="rp")
            l0, l1 = 0, nl
            if b == 0:
                l0 = 1                      # re = -1 is a zero row pair
                nc.gpsimd.memset(rp[0:8, :].bitcast(DT_F32), 0.0)
            if b == NB - 1:
                l1 = nl - 1                 # re = 128 is a zero row pair
                # 32-aligned base; rows below (nl-1)*8 are re-loaded by
                # the DMA below, which follows in program order (WAW)
                nc.gpsimd.memset(rp[64:128, :].bitcast(DT_F32), 0.0)
            re0 = h0 - 1 + l0
            asrc = apair[re0:re0 + (l1 - l0)]
            if b == 0:
                # stage the B=0 windows (both parities) first so the first
                # transposes can start as early as possible
                nc.sync.dma_start(out=rp[l0 * 8:l1 * 8, 0:128],
                                  in_=asrc[:, :, 0:128])
                nc.sync.dma_start(out=rp[l0 * 8:l1 * 8, 768:896],
                                  in_=asrc[:, :, 768:896])
                nc.sync.dma_start(out=rp[l0 * 8:l1 * 8, 128:768],
                                  in_=asrc[:, :, 128:768])
                nc.sync.dma_start(out=rp[l0 * 8:l1 * 8, 896:2 * RW],
                                  in_=asrc[:, :, 896:2 * RW])
            else:
                nc.sync.dma_start(out=rp[l0 * 8:l1 * 8, :], in_=asrc)

            oacc = opool.tile([128, OUTROW], DT_F32, tag="oacc")
            for B in range(NWB):
                win, kw, nmm, ncols = BGEO[B]
                # rhs weights for the four fh taps (B=0/6 use row slices)
                r0 = win - (114 * B - 6)         # 6 at B=0 else 0
                if B == 0:
                    wrhs0 = wbe0[0][0:kw + 1, 0:nmm]
                    wrhs = [wbe0[fh][0:kw, 0:nmm] for fh in range(1, F)]
                elif B == 6:
                    wrhs0 = wbe6[0:kw + 1, 0:nmm]
                    wrhs = [wb_sb[fh][0:kw, 0:nmm] for fh in range(1, F)]
                else:
                    wrhs0 = wb_sb[0][0:kw + 1, 0:nmm]
                    wrhs = [wb_sb[fh][0:kw, 0:nmm] for fh in range(1, F)]
                ones_base = (kw // 32) * 32      # 32-aligned memset base
                tsb = []
                for p in range(2):
                    ptr = pt_pool.tile([kw, 128], dt_mm, tag="ptr", name="ptr")
                    nc.tensor.transpose(
                        ptr[:], rp[:, RW * p + win:RW * p + win + kw],
                        ident_ap)
                    t = tpool.tile([KB, 128], dt_mm, tag=f"t{p}", name=f"tsb{p}")
                    if p == 0:
                        # ones row at partition kw (bias): memset a 32-aligned
                        # range; the evict below overwrites rows < kw
                        nc.gpsimd.memset(
                            t[ones_base:kw + 1, :].bitcast(DT_F32), 1.0)
                    nc.vector.tensor_copy(t[0:kw, :], ptr[:])
                    tsb.append(t)
                pc = pc_pool.tile([128, NMM], DT_F32, tag="pc")
                nc.tensor.matmul(pc[0:m, 0:nmm], tsb[0][0:kw + 1, 0:m],
                                 wrhs0, start=True, stop=False)
                nc.tensor.matmul(pc[0:m, 0:nmm], tsb[1][0:kw, 0:m],
                                 wrhs[0], start=False, stop=False)
                nc.tensor.matmul(pc[0:m, 0:nmm], tsb[0][0:kw, 8:8 + m],
                                 wrhs[1], start=False, stop=False)
                nc.tensor.matmul(pc[0:m, 0:nmm], tsb[1][0:kw, 8:8 + m],
                                 wrhs[2], start=False, stop=True)
                # ReLU eviction: alternate ACT/DVE to balance engines
                if B % 3 == 2:
                    nc.scalar.activation(
                        oacc[0:m, 304 * B:304 * B + ncols], pc[0:m, 0:ncols],
                        mybir.ActivationFunctionType.Relu)
                else:
                    nc.vector.tensor_scalar_max(
                        oacc[0:m, 304 * B:304 * B + ncols],
                        pc[0:m, 0:ncols], 0.0)

            dst = z_out[:, h0:h0 + nh, :].rearrange("i j c -> j i c")
            nc.scalar.dma_start(out=dst, in_=oacc[0:m, :])

    _split_multi_waits(nc)
    return nc


_NC_CACHE = {}


def _get_nc(dt_mm=DT_MM):
    key = str(dt_mm)
    if key not in _NC_CACHE:
        _NC_CACHE[key] = _build_nc(dt_mm)
    return _NC_CACHE[key]


def kernel(A_prev, W, b, _trace=False, _dt=None):
    A_prev = np.ascontiguousarray(A_prev, dtype=np.float32)
    W = np.asarray(W, dtype=np.float32)
    b = np.asarray(b, dtype=np.float32)
    comb, ident = _make_wband(W, b)

    nc = _get_nc(_dt or DT_MM)
    in_maps = []
    for c in range(N_CORES):
        shard = A_prev[c * IMG:(c + 1) * IMG].reshape(IMG, H, RW)
        in_maps.append({"A": shard, "consts": comb, "ident": ident})

    res = run_bass_kernel_spmd(nc, in_maps, list(range(N_CORES)),
                               trace=_trace)
    out = np.concatenate([res.results[c]["Z"].reshape(IMG, -1)
                          for c in range(N_CORES)], axis=0)
    if _trace:
        return out, res
    return out
